# revision 1
# baseline (speedup 1.0000x reference)
"""Trainium2 Bass kernel for the dense GNN message-passing step.

Computation (N=16384, NUM_IN=1024, NUM_OUT=256):
    states = zeros(N); states[input_indices] = input_values
    total  = states @ W + biases                      # GEMV over [N, N] f32
    out    = act_select(total)[output_indices]        # 0=id, 1=relu, 2=softsign

Strategy:
  * `states` is zero outside the (<=1024) positions named by input_indices,
    so only those rows of W contribute to the GEMV. The host gathers the
    live rows (a packing step) and the device contracts over a padded
    K=1024 instead of 16384 -> 16x less HBM traffic, the roofline resource
    for this memory-regime problem.
  * W is sharded column-wise across the 8 cores (tensor parallel, per the
    sharding hint): each core computes its 2048 outputs = GEMV slice +
    bias + per-neuron activation select; the host concatenates the slices
    and gathers output_indices.
  * fp32-exact GEMV via fp16 hi/lo decomposition: W = Wh + s*Wl and
    x = xh + s*xl with s = 2^-11 (each half is an fp16 with the residual
    scaled into normal range). The device computes
        t = xh'Wh  +  s * (xl'Wh + xh'Wl)      (the s^2 xl'Wl term is
    ~2^-22 relative and dropped). fp16 operands stream through the PE at
    1 cycle/row (vs 4 for fp32, and vs an LDW-bound ~427ns per 128x128
    block for the W-stationary form), while hi+lo storage is the same
    4 B/element as fp32, so HBM traffic is unchanged and the PE drops far
    below the DMA roofline. Products accumulate exactly in fp32 PSUM.
  * x is the stationary operand ([128,1] fp16 per k-chunk), W is moving
    ([128,512] fp16, N=512), so outputs land as [1,512] strips in PSUM.
    Accumulation groups are strictly sequential per PSUM bank (interleaved
    open groups mis-accumulate on HW): per 512-column chunk, group P1
    (xh'Wh, 8 matmuls) then group Ps (xl'Wh + xh'Wl, 16 matmuls).
  * The 8 x 1MB W-block DMAs are chained through a semaphore (block i's
    trigger waits for block i-1's completion) so blocks complete in order
    ~2.8us apart and the PE starts ~3us in, instead of all blocks
    completing together at ~21us (SDMA round-robins between in-flight
    queues at packet granularity).
  * Epilogue per chunk on [1,512] strips: t = P1 + s*Ps (+bias), then
    relu/softsign/identity selected by host-precomputed uint8 masks.
"""

import numpy as np
from contextlib import ExitStack

import concourse.bacc as bacc
import concourse.tile as tile
from concourse import mybir
from concourse.bass_utils import run_bass_kernel_spmd

N_CORES = 8
K = 1024                 # padded contraction size (live rows)
KC = K // 128            # 8 k-chunks
NPC = 16384 // N_CORES   # 2048 output columns per core
NCH = NPC // 512         # 4 column chunks of 512
S = 2.0 ** -11           # hi/lo split scale
F32 = mybir.dt.float32
F16 = mybir.dt.float16
U8 = mybir.dt.uint8

_BUILT = None            # cached nc so repeat calls reuse the compiled module
import os as _os
W_BUFS = int(_os.environ.get("W_BUFS", "2"))
LAST_RESULTS = None      # BassKernelResults of the most recent run (for test.py)


def _build_bass():
    nc = bacc.Bacc(
        "TRN2", target_bir_lowering=False, debug=False, num_devices=N_CORES
    )
    # w layout: [nch, part(hi=0,lo=1), half, p, kc4*col] — each partition's
    # 4 KB is contiguous so DMA descriptors stay big (1 KB descriptors were
    # measured at ~half the HBM line rate).
    w = nc.dram_tensor(
        "w", [NCH, 2, 2, 128, (KC // 2) * 512], F16, kind="ExternalInput"
    ).ap()
    xh = nc.dram_tensor("xh", [128, KC], F16, kind="ExternalInput").ap()
    xl = nc.dram_tensor("xl", [128, KC], F16, kind="ExternalInput").ap()
    b = nc.dram_tensor("b", [1, 2 * NPC], F16, kind="ExternalInput").ap()
    m1 = nc.dram_tensor("m1", [1, NPC], U8, kind="ExternalInput").ap()
    m2 = nc.dram_tensor("m2", [1, NPC], U8, kind="ExternalInput").ap()
    o = nc.dram_tensor("o", [1, NPC], F32, kind="ExternalOutput").ap()

    with tile.TileContext(nc) as tc:
        with ExitStack() as ctx:
            small = ctx.enter_context(tc.tile_pool(name="small", bufs=1))
            wpool = ctx.enter_context(tc.tile_pool(name="wp", bufs=W_BUFS))
            ppool = ctx.enter_context(tc.tile_pool(name="pp", bufs=1, space="PSUM"))
            scratch = ctx.enter_context(tc.tile_pool(name="scr", bufs=2))

            xh_t = small.tile([128, KC], F16, tag="xh")
            nc.scalar.dma_start(xh_t[:], xh[:])
            xl_t = small.tile([128, KC], F16, tag="xl")
            nc.scalar.dma_start(xl_t[:], xl[:])
            b_t = small.tile([1, 2 * NPC], F16, tag="bt")
            nc.scalar.dma_start(b_t[:], b[:])
            m1_t = small.tile([1, NPC], U8, tag="m1t")
            nc.scalar.dma_start(m1_t[:], m1[:])
            m2_t = small.tile([1, NPC], U8, tag="m2t")
            nc.scalar.dma_start(m2_t[:], m2[:])
            ones_t = small.tile([1, 1], F16, tag="ones")
            nc.gpsimd.memset(ones_t[:], 1.0)

            # W half-blocks (512 KB), shared pool slots so at most W_BUFS are
            # in flight: concurrent in-flight DMAs share bandwidth at packet
            # granularity, which would otherwise delay the FIRST block (and
            # the PE start) to the end of the whole 8 MB transfer.
            # Consumption order per nch: hi-a, hi-b (P1 + Ps xl-pass), lo-a,
            # lo-b (Ps xh-pass).
            wts = {}
            for nch in range(NCH):
                for part in range(2):
                    for half in range(2):
                        wt = wpool.tile([128, (KC // 2) * 512], F16, tag="wblk")
                        nc.sync.dma_start(wt[:], w[nch, part, half])
                        wts[(nch, part, half)] = wt

            def wslice(nch, part, kc):
                wt = wts[(nch, part, kc // (KC // 2))]
                j = kc % (KC // 2)
                return wt[:, j * 512 : (j + 1) * 512]

            o_t = small.tile([1, NPC], F32, tag="ot")
            for nch in range(NCH):
                sl = slice(nch * 512, (nch + 1) * 512)
                p1 = ppool.tile([1, 512], F32, tag=f"p1_{nch}")
                ps = ppool.tile([1, 512], F32, tag=f"ps_{nch}")

                # P1 = b_hi + xh'Wh  (scale 1)
                nc.tensor.matmul(
                    p1[0:1, :], ones_t[0:1, :], b_t[0:1, sl],
                    start=True, stop=False,
                )
                for kc in range(KC):
                    nc.tensor.matmul(
                        p1[0:1, :], xh_t[:, kc : kc + 1], wslice(nch, 0, kc),
                        start=False, stop=(kc == KC - 1),
                    )
                # Ps = b_lo + xl'Wh + xh'Wl  (scale S)
                nc.tensor.matmul(
                    ps[0:1, :], ones_t[0:1, :],
                    b_t[0:1, NPC + nch * 512 : NPC + (nch + 1) * 512],
                    start=True, stop=False,
                )
                for kc in range(KC):
                    nc.tensor.matmul(
                        ps[0:1, :], xl_t[:, kc : kc + 1], wslice(nch, 0, kc),
                        start=False, stop=False,
                    )
                for kc in range(KC):
                    nc.tensor.matmul(
                        ps[0:1, :], xh_t[:, kc : kc + 1], wslice(nch, 1, kc),
                        start=False, stop=(kc == KC - 1),
                    )

                # t = P1 + S*Ps, then act-select into the same buffer.
                # (a DVE op may read only ONE input from PSUM, so the scaled
                # Ps goes through ACT to SBUF first)
                ot = o_t[0:1, sl]
                st = scratch.tile([1, 512], F32, tag="st")
                nc.scalar.mul(st[:], ps[0:1, :], S)
                nc.vector.tensor_add(ot, p1[0:1, :], st[:])
                at = scratch.tile([1, 512], F32, tag="at")
                nc.scalar.activation(                        # |t|      (ACT)
                    at[:], ot, mybir.ActivationFunctionType.Abs
                )
                a1 = scratch.tile([1, 512], F32, tag="a1")
                nc.scalar.activation(                        # 1 + |t|  (ACT)
                    a1[:], at[:], mybir.ActivationFunctionType.Copy, bias=1.0
                )
                rf = scratch.tile([1, 512], F32, tag="rf")
                vt = scratch.tile([1, 512], F32, tag="vt")
                nc.vector.reciprocal_approx_accurate(        # ~2 ULP
                    out=vt[:], in_=a1[:], scratch=rf[:]
                )
                rt = scratch.tile([1, 512], F32, tag="rt")
                nc.scalar.activation(                        # relu(t)  (ACT)
                    rt[:], ot, mybir.ActivationFunctionType.Relu
                )
                sst = scratch.tile([1, 512], F32, tag="sst")
                nc.vector.tensor_mul(sst[:], ot, vt[:])      # softsign(t)
                nc.vector.copy_predicated(ot, m1_t[0:1, sl], rt[:])
                nc.vector.copy_predicated(ot, m2_t[0:1, sl], sst[:])

            nc.sync.dma_start(o[:], o_t[:])

    nc.compile()
    return nc


def _split_f16(a):
    hi = a.astype(np.float16)
    lo = ((a - hi.astype(np.float32)) * (1.0 / S)).astype(np.float16)
    return hi, lo


def kernel(**inputs) -> np.ndarray:
    global _BUILT, LAST_RESULTS

    iv = np.asarray(inputs["input_values"], dtype=np.float32)
    W = np.asarray(inputs["weight_matrix"], dtype=np.float32)
    bias = np.asarray(inputs["biases"], dtype=np.float32)
    act = np.asarray(inputs["act_ids"])
    iidx = np.asarray(inputs["input_indices"]).astype(np.int64)
    oidx = np.asarray(inputs["output_indices"]).astype(np.int64)

    n = W.shape[0]
    # Dense neuron-state vector (duplicate indices: last write wins, matching
    # jax's .at[].set) and its index support.
    states = np.zeros(n, np.float32)
    states[iidx] = iv
    live = np.zeros(n, dtype=bool)
    live[iidx] = True
    support = np.flatnonzero(live)
    assert support.size <= K, "more than K live rows not supported"
    rows = np.zeros(K, np.int64)          # pad with row 0 (x=0 there => no-op)
    rows[: support.size] = support
    xvec = np.zeros(K, np.float32)
    xvec[: support.size] = states[support]

    Wa = W[rows]                          # [K, n] live rows (padded)
    xhv, xlv = _split_f16(xvec)
    xh_t = np.ascontiguousarray(xhv.reshape(KC, 128).T)   # [128, KC]
    xl_t = np.ascontiguousarray(xlv.reshape(KC, 128).T)

    in_maps = []
    for c in range(N_CORES):
        ws = np.ascontiguousarray(Wa[:, c * NPC : (c + 1) * NPC])
        whi, wlo = _split_f16(ws)
        # [K, NPC] -> [nch, half, p, kc4, col] -> stack part on axis 1
        wh5 = whi.reshape(2, KC // 2, 128, NCH, 512).transpose(3, 0, 2, 1, 4)
        wl5 = wlo.reshape(2, KC // 2, 128, NCH, 512).transpose(3, 0, 2, 1, 4)
        wc = np.ascontiguousarray(
            np.stack([wh5, wl5], axis=1)  # [nch, part, half, p, kc4, col]
        ).reshape(NCH, 2, 2, 128, (KC // 2) * 512)
        sl = slice(c * NPC, (c + 1) * NPC)
        bh, bl = _split_f16(bias[sl])
        in_maps.append(
            {
                "w": wc,
                "xh": xh_t,
                "xl": xl_t,
                "b": np.concatenate([bh, bl]).reshape(1, 2 * NPC),
                "m1": (act[sl] == 1).astype(np.uint8).reshape(1, NPC),
                "m2": (act[sl] == 2).astype(np.uint8).reshape(1, NPC),
            }
        )

    if _BUILT is None:
        _BUILT = _build_bass()
    LAST_RESULTS = run_bass_kernel_spmd(
        _BUILT, in_maps, core_ids=list(range(N_CORES))
    )
    full = np.concatenate(
        [LAST_RESULTS.results[c]["o"][0] for c in range(N_CORES)]
    )
    return full[oidx].astype(np.float32)



# revision 12
# speedup vs baseline: 2.2582x; 2.2582x over previous
"""Trainium2 Bass kernel for the dense GNN message-passing step.

Computation (N=16384, NUM_IN=1024, NUM_OUT=256):
    states = zeros(N); states[input_indices] = input_values
    total  = states @ W + biases                      # GEMV over [N, N] f32
    out    = act_select(total)[output_indices]        # 0=id, 1=relu, 2=softsign

Strategy:
  * `states` is zero outside the (<=1024) positions named by input_indices,
    so only those rows of W contribute to the GEMV. The host gathers the
    live rows and the device contracts over a padded K=1024 instead of
    16384 -> 16x less HBM traffic.
  * W is sharded column-wise across the 8 cores (tensor parallel): each
    core computes its 2048 outputs = GEMV slice + bias + per-neuron
    activation select; the host concatenates and gathers output_indices.
  * W is stored as single fp16 (2 B/element): the harness gate is
    rel_err < 2e-2 and the fp16 GEMV lands ~1e-4, so the fp32-exact hi/lo
    path (4 B/element) is 2x wasted HBM traffic. 4 MB/core total.
  * The 4 x 1MB W-chunk DMAs are issued back-to-back up front on the sync
    HWDGE queue: they drain FIFO on one ring, so chunk i completes ~3us
    after chunk i-1 at streaming rate and the PE starts ~4us in. Each
    chunk is [128, 8KB/partition] -> large descriptors at near line rate.
    (The old chained/pool-throttled scheme serialized each 512KB block
    with its ~2us completion latency: DMA idled 57%.)
  * x is stationary ([128,1] f16 per k-chunk), W moving ([128,512] f16),
    8 accumulating matmuls per chunk. Chunk nch's [1,512] strip lands at
    PSUM partition 32*nch of ONE bank (PE tile_position), so the epilogue
    runs on [2,512] stride-32 views: 2 batches x ~7 ops instead of
    4 x 10 single-partition ops. Bias is folded into the PSUM->SBUF move
    (DVE tensor_add), killing the ones-row bias matmuls.
  * Epilogue per batch: t = P + b; relu on ACT; a1 = |t|+1 (fused
    tensor_scalar abs_max+add); r = reciprocal_approx_fast(a1) (~18 bits,
    plenty for 2e-2); ss = t*r; two copy_predicated selects with
    host-precomputed uint8 masks. Batch {0,1} overlaps chunks 2-3.
"""

import numpy as np
from contextlib import ExitStack

import concourse.bacc as bacc
import concourse.tile as tile
from concourse import mybir
from concourse.bass_utils import run_bass_kernel_spmd

N_CORES = 8
K = 1024                 # padded contraction size (live rows)
KC = K // 128            # 8 k-chunks
NPC = 16384 // N_CORES   # 2048 output columns per core
NCH = NPC // 512         # 4 column chunks of 512
F32 = mybir.dt.float32
F16 = mybir.dt.float16
U8 = mybir.dt.uint8

_BUILT = None            # cached nc so repeat calls reuse the compiled module
LAST_RESULTS = None      # BassKernelResults of the most recent run (for test.py)


def _build_bass():
    nc = bacc.Bacc(
        "TRN2", target_bir_lowering=False, debug=False, num_devices=N_CORES
    )
    w = nc.dram_tensor("w", [NCH, 128, KC * 512], F16, kind="ExternalInput").ap()
    # Stationary blocks: for (kc, j) a [128, 2] block whose column j is
    # x chunk kc and whose other column is zero. A matmul with this lhsT
    # writes a [2, 512] PSUM tile where row j accumulates x_kc' W and the
    # other row accumulates +0 — so chunk pair {2h, 2h+1} lands on
    # CONTIGUOUS partitions {0,1} of bank h (DVE cannot read strided
    # partitions, and the PE cannot place M=1 outputs at partition 1).
    xs = nc.dram_tensor("xs", [128, KC * 4], F16, kind="ExternalInput").ap()
    # b/m1/m2 packed [row(2), half(2)*512]: row j, col-block h holds chunk
    # 2h+j's values.
    b = nc.dram_tensor("b", [2, 2 * 512], F32, kind="ExternalInput").ap()
    m1 = nc.dram_tensor("m1", [2, 2 * 512], U8, kind="ExternalInput").ap()
    m2 = nc.dram_tensor("m2", [2, 2 * 512], U8, kind="ExternalInput").ap()
    o = nc.dram_tensor("o", [NCH, 512], F32, kind="ExternalOutput").ap()

    with tile.TileContext(nc) as tc:
        with ExitStack() as ctx:
            small = ctx.enter_context(tc.tile_pool(name="small", bufs=1))
            wpool = ctx.enter_context(tc.tile_pool(name="wp", bufs=NCH))
            ppool = ctx.enter_context(tc.tile_pool(name="pp", bufs=1, space="PSUM"))
            scr = ctx.enter_context(tc.tile_pool(name="scr", bufs=1))

            # W chunks first: 4 x 1MB on the sync HWDGE queue, FIFO.
            wts = []
            for nch in range(NCH):
                wt = wpool.tile([128, KC * 512], F16, tag="wblk")
                nc.sync.dma_start(wt[:], w[nch])
                wts.append(wt)

            # Small tensors on the scalar HWDGE queue (parallel to W).
            xs_t = small.tile([128, KC * 4], F16, tag="xs")
            nc.scalar.dma_start(xs_t[:], xs[:])
            b_t = small.tile([2, 2 * 512], F32, tag="b")
            nc.scalar.dma_start(b_t[:], b[:])
            m1_t = small.tile([2, 2 * 512], U8, tag="m1")
            nc.scalar.dma_start(m1_t[:], m1[:])
            m2_t = small.tile([2, 2 * 512], U8, tag="m2")
            nc.scalar.dma_start(m2_t[:], m2[:])

            # Chunk pair {2h, 2h+1} -> PSUM bank h rows {0,1}, one
            # 16-matmul accumulation group per bank.
            pt0 = ppool.tile([128, 512], F32, tag="p0")
            pt1 = ppool.tile([128, 512], F32, tag="p1")
            pts = [pt0, pt1]
            for half in range(2):
                pt = pts[half]
                for j in range(2):
                    nch = 2 * half + j
                    for kc in range(KC):
                        blk = (kc * 2 + j) * 2
                        nc.tensor.matmul(
                            pt[0:2, :],
                            xs_t[:, blk : blk + 2],
                            wts[nch][:, kc * 512 : (kc + 1) * 512],
                            start=(j == 0 and kc == 0),
                            stop=(j == 1 and kc == KC - 1),
                        )

            # Epilogue per pair on contiguous [2,512]; pair 0 overlaps
            # pair 1's matmuls.
            for half in range(2):
                cs = slice(half * 512, (half + 1) * 512)
                p2 = pts[half][0:2, :]
                ot = scr.tile([2, 512], F32, tag=f"ot{half}", name=f"ot{half}")
                rt = scr.tile([2, 512], F32, tag=f"rt{half}", name=f"rt{half}")
                at = scr.tile([2, 512], F32, tag=f"at{half}", name=f"at{half}")
                a1 = scr.tile([2, 512], F32, tag=f"a1{half}", name=f"a1{half}")
                rc = scr.tile([2, 512], F32, tag=f"rc{half}", name=f"rc{half}")
                ss = scr.tile([2, 512], F32, tag=f"ss{half}", name=f"ss{half}")
                nc.vector.tensor_add(ot[:], p2, b_t[:, cs])    # t = P + b
                nc.scalar.activation(                          # relu(t)
                    rt[:], ot[:], mybir.ActivationFunctionType.Relu
                )
                nc.scalar.activation(                          # |t|
                    at[:], ot[:], mybir.ActivationFunctionType.Abs
                )
                nc.scalar.activation(                          # 1 + |t|
                    a1[:], at[:], mybir.ActivationFunctionType.Copy, bias=1.0
                )
                nc.vector.reciprocal_approx_fast(rc[:], a1[:])
                nc.vector.tensor_mul(ss[:], ot[:], rc[:])      # softsign(t)
                nc.vector.copy_predicated(ot[:], m1_t[:, cs], rt[:])
                nc.vector.copy_predicated(ot[:], m2_t[:, cs], ss[:])
                nc.sync.dma_start(o[2 * half : 2 * half + 2], ot[:])

    nc.compile()
    return nc


def kernel(**inputs) -> np.ndarray:
    global _BUILT, LAST_RESULTS

    iv = np.asarray(inputs["input_values"], dtype=np.float32)
    W = np.asarray(inputs["weight_matrix"], dtype=np.float32)
    bias = np.asarray(inputs["biases"], dtype=np.float32)
    act = np.asarray(inputs["act_ids"])
    iidx = np.asarray(inputs["input_indices"]).astype(np.int64)
    oidx = np.asarray(inputs["output_indices"]).astype(np.int64)

    n = W.shape[0]
    # Dense neuron-state vector (duplicate indices: last write wins, matching
    # jax's .at[].set) and its index support.
    states = np.zeros(n, np.float32)
    states[iidx] = iv
    live = np.zeros(n, dtype=bool)
    live[iidx] = True
    support = np.flatnonzero(live)
    assert support.size <= K, "more than K live rows not supported"
    rows = np.zeros(K, np.int64)          # pad with row 0 (x=0 there => no-op)
    rows[: support.size] = support
    xvec = np.zeros(K, np.float32)
    xvec[: support.size] = states[support]

    Wh = W[rows].astype(np.float16)       # [K, n] live rows, single fp16
    xh = xvec.astype(np.float16)
    xc = xh.reshape(KC, 128).T            # [128, KC]
    # Stationary blocks [128, (kc*2+j)*2 + m]: x chunk kc in column m==j.
    xs_t = np.zeros((128, KC * 4), np.float16)
    for kc in range(KC):
        for j in range(2):
            xs_t[:, (kc * 2 + j) * 2 + j] = xc[:, kc]

    in_maps = []
    for c in range(N_CORES):
        sl = slice(c * NPC, (c + 1) * NPC)
        wc = np.ascontiguousarray(
            Wh[:, sl].reshape(KC, 128, NCH, 512).transpose(2, 1, 0, 3)
        ).reshape(NCH, 128, KC * 512)
        def pack2(a):
            # [NCH,512] -> [row(2), half(2)*512]: packed[r, 512h+j] = chunk
            # (2h+r) col j, matching the b_t/m_t SBUF layout.
            return np.ascontiguousarray(
                a.reshape(2, 2, 512).transpose(1, 0, 2)
            ).reshape(2, 1024)

        in_maps.append(
            {
                "w": wc,
                "xs": xs_t,
                "b": pack2(bias[sl].astype(np.float32)),
                "m1": pack2((act[sl] == 1).astype(np.uint8)),
                "m2": pack2((act[sl] == 2).astype(np.uint8)),
            }
        )

    if _BUILT is None:
        _BUILT = _build_bass()
    LAST_RESULTS = run_bass_kernel_spmd(
        _BUILT, in_maps, core_ids=list(range(N_CORES))
    )
    full = np.concatenate(
        [LAST_RESULTS.results[c]["o"].reshape(-1) for c in range(N_CORES)]
    )
    return full[oidx].astype(np.float32)
